# revision 7
# baseline (speedup 1.0000x reference)
import os
import numpy as np

# ---- problem constants (hardcoded; kernel.py must be self-contained) ----
IMG, WS, SHIFT = 32, 8, 4
C, HEADS, DEPTH = 512, 16, 24
E_DIM, N_E, B = 256, 8192, 8
L = IMG * IMG            # 1024
NW = WS * WS             # 64 tokens per window
HD = C // HEADS          # 32
NWIN = (IMG // WS) ** 2  # 16
FH = 4 * C               # 2048
P = 128
VBLK = HD + 2            # 34 (32 vals + softmax-denominator col + pad)
VW = HEADS * VBLK        # 544
NT = L // P              # 8 token tiles
KC = C // P              # 4 k-tiles over C
KE = E_DIM // P          # 2 k-tiles over E_DIM
SCALE = HD ** -0.5

_DEPTH = int(os.environ.get("BT_DEPTH", DEPTH))
_NCORES = int(os.environ.get("BT_NCORES", 8))
_STAGE = int(os.environ.get("BT_STAGE", 99))  # debug bisect ladder


# ---- host-side helpers (mirror reference.py) ----
def _rel_index():
    coords = np.stack(np.meshgrid(np.arange(WS), np.arange(WS), indexing='ij'))
    cf = coords.reshape(2, -1)
    rel = (cf[:, :, None] - cf[:, None, :]).transpose(1, 2, 0)
    rel[:, :, 0] += WS - 1
    rel[:, :, 1] += WS - 1
    rel[:, :, 0] *= 2 * WS - 1
    return rel.sum(-1)  # [NW, NW] int


def _shift_mask():
    img = np.zeros((IMG, IMG), np.float32)
    cnt = 0
    sl = (slice(0, -WS), slice(-WS, -SHIFT), slice(-SHIFT, None))
    for hs in sl:
        for ws_ in sl:
            img[hs, ws_] = cnt
            cnt += 1
    win = img.reshape(IMG // WS, WS, IMG // WS, WS).transpose(0, 2, 1, 3).reshape(-1, NW)
    diff = win[:, None, :] - win[:, :, None]
    return np.where(diff != 0, -100.0, 0.0).astype(np.float32)  # [NWIN, NW, NW]


def _win_perm():
    """raster token index -> window-major position; perm[t_raster] = t_dev"""
    t = np.arange(L).reshape(IMG, IMG)
    wm = t.reshape(IMG // WS, WS, IMG // WS, WS).transpose(0, 2, 1, 3).reshape(-1)
    inv = np.empty(L, np.int64)
    inv[wm] = np.arange(L)
    return wm, inv  # wm: dev->raster, inv: raster->dev


_WM, _WM_INV = _win_perm()
_REL = _rel_index()
_MASK = _shift_mask()

_HAS_BIAS = False  # set by _prepare; device build skips rank-1 bias matmuls if False


def _prepare(inputs):
    import ml_dtypes
    global _HAS_BIAS
    BF = ml_dtypes.bfloat16
    f32 = lambda a: np.ascontiguousarray(a, dtype=np.float32)
    bf = lambda a: np.ascontiguousarray(np.asarray(a, np.float32), dtype=BF)
    x = np.asarray(inputs['x'], np.float32)          # [B, L, E]
    dec_w = np.asarray(inputs['dec_w'], np.float32)  # [C, E]
    dec_b = np.asarray(inputs['dec_b'], np.float32)
    pos = np.asarray(inputs['pos_embed'], np.float32)[0]  # [L, C]
    n1w = np.asarray(inputs['n1w'], np.float32)
    n1b = np.asarray(inputs['n1b'], np.float32)
    qkv_w = np.asarray(inputs['qkv_w'], np.float32)
    qkv_b = np.asarray(inputs['qkv_b'], np.float32)
    proj_w = np.asarray(inputs['proj_w'], np.float32)
    proj_b = np.asarray(inputs['proj_b'], np.float32)
    rel_bias = np.asarray(inputs['rel_bias'], np.float32)
    n2w = np.asarray(inputs['n2w'], np.float32)
    n2b = np.asarray(inputs['n2b'], np.float32)
    fc1_w = np.asarray(inputs['fc1_w'], np.float32)
    fc1_b = np.asarray(inputs['fc1_b'], np.float32)
    fc2_w = np.asarray(inputs['fc2_w'], np.float32)
    fc2_b = np.asarray(inputs['fc2_b'], np.float32)
    normf_w = np.asarray(inputs['normf_w'], np.float32)
    normf_b = np.asarray(inputs['normf_b'], np.float32)
    pred_w = np.asarray(inputs['pred_w'], np.float32)
    pred_b = np.asarray(inputs['pred_b'], np.float32)

    D = _DEPTH
    sh = {}
    sh['decw'] = f32(dec_w.T)                                    # [E, C]
    sh['posb'] = f32((pos + dec_b[None, :])[_WM])                # [L, C]

    wqk = np.empty((D, C, 2 * C), BF)
    bqk = np.zeros((D, P, 8), np.float32)
    wvp = np.zeros((D, C, VW), np.float32)
    vb = np.zeros((D, P, VW), np.float32)
    ab = np.empty((D, NT, P, 4, 4, NW), np.float32)  # (wp2, 2w*tk, rg, hi, tq)
    wp_ = np.empty((D, C, C), BF)
    pbr = np.empty((D, 1, C), BF)
    f1 = np.empty((D, C, FH), BF)
    f1b = np.empty((D, P, FH // P), np.float32)
    f2 = np.empty((D, FH, C), BF)
    f2br = np.empty((D, 1, C), BF)

    for i in range(D):
        Wm = qkv_w[i] * n1w[i][None, :]           # [3C, C]
        bm = qkv_w[i] @ n1b[i] + qkv_b[i]         # [3C]
        Wm = Wm.copy()
        bm = bm.copy()
        Wm[:C] *= SCALE
        bm[:C] *= SCALE
        wqk[i] = Wm[:2 * C].T.astype(BF)          # [C, 2C]
        bqk[i] = bm[:2 * C].reshape(8, P).T       # bias for out-channel tile mo at [:, mo]
        # v with padded 34-blocks; ones column + bias via vb (DVE add on copy-out)
        for h in range(HEADS):
            wvp[i][:, h * VBLK:h * VBLK + HD] = Wm[2 * C + h * HD:2 * C + (h + 1) * HD].T
            vb[i][:, h * VBLK:h * VBLK + HD] = bm[2 * C + h * HD:2 * C + (h + 1) * HD][None, :]
            vb[i][:, h * VBLK + HD] = 1.0
        # attention additive bias [tk, tq] per (win, head)
        bias = rel_bias[i][_REL]                  # [tq, tk, HEADS]
        shift = (i % 2) == 1
        for w in range(NWIN):
            for h in range(HEADS):
                a = bias[:, :, h].T               # [tk, tq]
                if shift:
                    a = a + _MASK[w].T
                wp2, w01 = w // 2, w % 2
                rg, hi = h % 4, h // 4
                ab[i, wp2, w01 * NW:(w01 + 1) * NW, rg, hi, :] = a
        wp_[i] = proj_w[i].T.astype(BF)
        pbr[i, 0] = proj_b[i].astype(BF)
        F1m = (fc1_w[i] * n2w[i][None, :]).T      # [C, FH]
        f1[i] = F1m.astype(BF)
        f1b[i] = (fc1_w[i] @ n2b[i] + fc1_b[i]).reshape(FH // P, P).T
        f2[i] = fc2_w[i].T.astype(BF)
        f2br[i, 0] = fc2_b[i].astype(BF)

    _HAS_BIAS = bool(np.any(proj_b != 0) or np.any(fc2_b != 0))

    sh['wqk'] = wqk
    sh['bqk'] = bqk
    sh['wvp'] = bf(wvp)
    sh['vb'] = vb
    sh['ab'] = np.ascontiguousarray(ab.reshape(D, NT, P, 16 * NW).astype(BF))
    sh['wp'] = wp_
    sh['pbr'] = pbr
    sh['f1'] = f1
    sh['f1b'] = f1b
    sh['f2'] = f2
    sh['f2br'] = f2br
    sh['nfw'] = f32(normf_w.reshape(KC, P).T)     # [P, KC]
    sh['nfb'] = f32(normf_b.reshape(KC, P).T)
    sh['pw'] = bf(pred_w.T)                       # [C, N_E]
    sh['pwb'] = f32(pred_b.reshape(N_E // P, P).T)  # [P, 64]
    # per-core xT in device token order: [E, L]
    xts = [np.ascontiguousarray(x[c][_WM].T) for c in range(B)]
    return sh, xts


# ---- device program ----
_BUILD_CACHE = {}


def _build(has_bias):
    key = (_DEPTH, has_bias)
    if key in _BUILD_CACHE:
        return _BUILD_CACHE[key]
    import concourse.bass as bass
    import concourse.mybir as mybir
    import concourse.tile as tile
    from concourse import bacc
    from concourse.masks import make_identity
    from contextlib import ExitStack

    F32 = mybir.dt.float32
    F32R = mybir.dt.float32r
    BF16 = mybir.dt.bfloat16
    AF = mybir.ActivationFunctionType
    ALU = mybir.AluOpType
    AX = mybir.AxisListType
    D = _DEPTH

    nc = bacc.Bacc("TRN2", target_bir_lowering=False, debug=False, num_devices=_NCORES)

    dr = {}
    def din(name, shape, dt):
        dr[name] = nc.dram_tensor(name, list(shape), dt, kind="ExternalInput").ap()
    din('xT', (E_DIM, L), F32R)
    din('decw', (E_DIM, C), F32R)
    din('posb', (L, C), F32)
    din('wqk', (D, C, 2 * C), BF16)
    din('bqk', (D, P, 8), F32)
    din('wvp', (D, C, VW), BF16)
    din('vb', (D, P, VW), F32)
    din('ab', (D, NT, P, 16 * NW), BF16)
    din('wp', (D, C, C), BF16)
    din('pbr', (D, 1, C), BF16)
    din('f1', (D, C, FH), BF16)
    din('f1b', (D, P, FH // P), F32)
    din('f2', (D, FH, C), BF16)
    din('f2br', (D, 1, C), BF16)
    din('nfw', (P, KC), F32)
    din('nfb', (P, KC), F32)
    din('pw', (C, N_E), BF16)
    din('pwb', (P, N_E // P), F32)
    outT = nc.dram_tensor("outT", [N_E, L], BF16, kind="ExternalOutput").ap()

    with tile.TileContext(nc) as tc, ExitStack() as ES:
        # ---------- persistent SBUF pools ----------
        cst = ES.enter_context(tc.tile_pool(name="cst", bufs=1))
        ident_f = cst.tile([P, P], F32)
        make_identity(nc, ident_f)
        ident_b = cst.tile([P, P], BF16)
        nc.scalar.copy(ident_b[:], ident_f[:])
        ones_col = cst.tile([1, P], BF16)
        nc.vector.memset(ones_col[:], 1.0)

        xp = ES.enter_context(tc.tile_pool(name="xp", bufs=1))
        hp = ES.enter_context(tc.tile_pool(name="hp", bufs=2))
        hTp = ES.enter_context(tc.tile_pool(name="hTp", bufs=2))
        qkp = ES.enter_context(tc.tile_pool(name="qkp", bufs=1))
        vp = ES.enter_context(tc.tile_pool(name="vp", bufs=1))
        attp = ES.enter_context(tc.tile_pool(name="attp", bufs=1))
        ppool = ES.enter_context(tc.tile_pool(name="ppool", bufs=2))
        abp = ES.enter_context(tc.tile_pool(name="abp", bufs=3))
        stp = ES.enter_context(tc.tile_pool(name="stp", bufs=2))
        recp = ES.enter_context(tc.tile_pool(name="recp", bufs=2))
        wqkp = ES.enter_context(tc.tile_pool(name="wqkp", bufs=2))
        wvpp = ES.enter_context(tc.tile_pool(name="wvpp", bufs=2))
        wpp = ES.enter_context(tc.tile_pool(name="wpp", bufs=2))
        f1p = ES.enter_context(tc.tile_pool(name="f1p", bufs=4))
        f2p = ES.enter_context(tc.tile_pool(name="f2p", bufs=4))
        gp = ES.enter_context(tc.tile_pool(name="gp", bufs=3))
        bp = ES.enter_context(tc.tile_pool(name="bp", bufs=2))
        outp = ES.enter_context(tc.tile_pool(name="outp", bufs=3))
        pwp = ES.enter_context(tc.tile_pool(name="pwp", bufs=4))

        x = xp.tile([P, NT, C], F32)

        # ---------- dec ----------
        with tc.tile_pool(name="decp", bufs=1) as decp, \
             tc.tile_pool(name="dps", bufs=2, space="PSUM") as dps:
            xT_sb = decp.tile([P, KE, L], F32R)
            nc.sync.dma_start(xT_sb[:], dr['xT'].rearrange("(k p) t -> p k t", p=P))
            decw_sb = decp.tile([P, KE, C], F32R)
            nc.sync.dma_start(decw_sb[:], dr['decw'].rearrange("(k p) c -> p k c", p=P))
            for tt in range(NT):
                pos_t = decp.tile([P, C], F32, name="pos_t", tag="pos", bufs=2)
                nc.sync.dma_start(pos_t[:], dr['posb'][tt * P:(tt + 1) * P, :])
                ps = dps.tile([P, C], F32)
                for kk in range(KE):
                    nc.tensor.matmul(ps[:], xT_sb[:, kk, tt * P:(tt + 1) * P],
                                     decw_sb[:, kk, :], start=(kk == 0), stop=(kk == KE - 1))
                nc.vector.tensor_add(x[:, tt], ps[:], pos_t[:])

        # ---------- layer-norm helper: bn_stats path, bf16 out ----------
        def layernorm(dst, src):
            for tt in range(NT):
                st6 = stp.tile([P, 6], F32, name="st6", tag=f"st6{tt % 2}")
                nc.vector.bn_stats(st6[:], src[:, tt])
                mv = stp.tile([P, 2], F32, name="mv", tag=f"mv{tt % 2}")
                nc.vector.bn_aggr(mv[:], st6[:])
                veps = stp.tile([P, 1], F32, name="veps", tag=f"veps{tt % 2}")
                nc.vector.tensor_scalar(veps[:], mv[:, 1:2], 1.0, 1e-5,
                                        ALU.mult, ALU.add)
                rstd = stp.tile([P, 1], F32, name="rstd", tag=f"rstd{tt % 2}")
                nc.scalar.activation(rstd[:], veps[:], AF.Sqrt)
                nc.vector.reciprocal(rstd[:], rstd[:])
                nb = stp.tile([P, 1], F32, name="nb", tag=f"nb{tt % 2}")
                nc.vector.tensor_scalar(nb[:], mv[:, 0:1], rstd[:, 0:1], -1.0,
                                        ALU.mult, ALU.mult)
                nc.scalar.activation(dst[:, tt], src[:, tt], AF.Identity,
                                     bias=nb[:], scale=rstd[:])

        # token-major [P, NT, C] bf16 -> C-major [P, KC, L] bf16 via DMA transpose
        def transpose_to(hT, src):
            for tt in range(NT):
                for ct in range(KC):
                    nc.sync.dma_start_transpose(
                        hT[:, ct, tt * P:(tt + 1) * P],
                        src[:, tt, ct * P:(ct + 1) * P])

        # shift permute between A-order and B-order (on C-major bf16 tiles)
        def permute(dstT, srcT, fwd):
            G = IMG // WS  # 4
            sv = srcT[:].rearrange("p k (a b i j) -> p k a b i j", a=G, b=G, i=WS, j=WS)
            dv = dstT[:].rearrange("p k (a b i j) -> p k a b i j", a=G, b=G, i=WS, j=WS)
            for qa in range(2):
                for qb in range(2):
                    di = slice(0, 4) if qa == 0 else slice(4, 8)
                    si = slice(4, 8) if qa == 0 else slice(0, 4)
                    dj = slice(0, 4) if qb == 0 else slice(4, 8)
                    sj = slice(4, 8) if qb == 0 else slice(0, 4)
                    for a in range(G):
                        sa = (a + qa) % G
                        if qb == 0:
                            bpairs = [(slice(0, G), slice(0, G))]
                        else:
                            bpairs = [(slice(0, G - 1), slice(1, G)), (slice(G - 1, G), slice(0, 1))]
                        for db, sb_ in bpairs:
                            for ct in range(KC):
                                eng = (nc.gpsimd, nc.vector)[(a + ct) % 2]
                                if fwd:
                                    eng.tensor_copy(dv[:, ct, a, db, di, dj],
                                                    sv[:, ct, sa, sb_, si, sj])
                                else:
                                    eng.tensor_copy(dv[:, ct, sa, sb_, si, sj],
                                                    sv[:, ct, a, db, di, dj])

        # ---------- layers ----------
        for i in range(D):
            if _STAGE < 1:
                break
            shift = (i % 2) == 1
            # LN1 -> h (bf16)
            h = hp.tile([P, NT, C], BF16, name="h")
            layernorm(h, x)
            hT_A = hTp.tile([P, KC, L], BF16, name="hT")
            transpose_to(hT_A, h)
            if shift:
                hT = hTp.tile([P, KC, L], BF16, name="hT")
                permute(hT, hT_A, True)
            else:
                hT = hT_A

            # qk + v
            if _STAGE < 2:
                continue
            wqk_sb = wqkp.tile([P, KC, 2 * C], BF16, name="wqk")
            nc.sync.dma_start(wqk_sb[:], dr['wqk'][i].rearrange("(k p) m -> p k m", p=P))
            bqk_sb = bp.tile([P, 8], F32, name="bqk", tag="bqk")
            nc.sync.dma_start(bqk_sb[:], dr['bqk'][i])
            qkT = qkp.tile([P, 8, L], BF16, name="qkT")
            wvp_sb = wvpp.tile([P, KC, VW], BF16, name="wvp")
            nc.sync.dma_start(wvp_sb[:], dr['wvp'][i].rearrange("(k p) m -> p k m", p=P))
            vb_sb = bp.tile([P, VW], F32, name="vb", tag="vb")
            nc.sync.dma_start(vb_sb[:], dr['vb'][i])
            v_aug = vp.tile([P, NT, VW], BF16, name="vaug")
            with tc.tile_pool(name="mmps1", bufs=2, space="PSUM") as mmps, \
                 tc.tile_pool(name="vps", bufs=2, space="PSUM") as vps:
                for mo in range(8):
                    ps = mmps.tile([P, L], F32, name="mm")
                    for kk in range(KC):
                        for tc2 in range(2):
                            nc.tensor.matmul(ps[:, tc2 * 512:(tc2 + 1) * 512],
                                             wqk_sb[:, kk, mo * P:(mo + 1) * P],
                                             hT[:, kk, tc2 * 512:(tc2 + 1) * 512],
                                             start=(kk == 0), stop=(kk == KC - 1),
                                             skip_group_check=True)
                    nc.scalar.activation(qkT[:, mo], ps[:], AF.Identity,
                                         bias=bqk_sb[:, mo:mo + 1])
                for tt in range(NT if _STAGE >= 3 else 0):
                    psv = vps.tile([P, VW], F32, name="vps")
                    for kk in range(KC):
                        nc.tensor.matmul(psv[:, 0:512], hT[:, kk, tt * P:(tt + 1) * P],
                                         wvp_sb[:, kk, 0:512], start=(kk == 0),
                                         stop=(kk == KC - 1), skip_group_check=True)
                        nc.tensor.matmul(psv[:, 512:VW], hT[:, kk, tt * P:(tt + 1) * P],
                                         wvp_sb[:, kk, 512:VW], start=(kk == 0),
                                         stop=(kk == KC - 1), skip_group_check=True)
                    nc.vector.tensor_add(v_aug[:, tt], psv[:], vb_sb[:])

            if _STAGE < 4:
                continue
            # attention: S + softmax + AV
            att = attp.tile([P, NT, C], BF16, name="att")
            with tc.tile_pool(name="sps", bufs=1, space="PSUM") as sps, \
                 tc.tile_pool(name="avps", bufs=1, space="PSUM") as avps:
                for wp2 in range(NT):
                    abt = abp.tile([P, 16 * NW], BF16, name=f"ab{wp2 % 3}")
                    nc.sync.dma_start(abt[:], dr['ab'][i, wp2])
                    pts = []
                    for rg in range(4):
                        sp = sps.tile([P, 4, NW], F32, name=f"s{rg}", tag=f"s{rg}")
                        spf = sp[:].rearrange("p a b -> p (a b)")
                        nc.tensor.matmul(spf, ident_b[:],
                                         abt[:, rg * 256:(rg + 1) * 256],
                                         start=True, stop=False, skip_group_check=True)
                        for hi in range(4):
                            for w01 in range(2):
                                qs = qkT[rg * HD:(rg + 1) * HD, hi,
                                         (wp2 * 2 + w01) * NW:(wp2 * 2 + w01 + 1) * NW]
                                ks = qkT[rg * HD:(rg + 1) * HD, 4 + hi,
                                         (wp2 * 2 + w01) * NW:(wp2 * 2 + w01 + 1) * NW]
                                nc.tensor.matmul(sp[w01 * NW:(w01 + 1) * NW, hi, :],
                                                 ks, qs, start=False,
                                                 stop=(hi == 3 and w01 == 1),
                                                 tile_position=(rg * HD, w01 * NW),
                                                 skip_group_check=True)
                        pt = ppool.tile([P, 4, NW], BF16, name=f"p{rg}", tag=f"p{rg}")
                        nc.scalar.activation(pt[:].rearrange("p a b -> p (a b)"),
                                             spf, AF.Exp)
                        pts.append(pt)
                    for w01 in range(2):
                        rows = slice(w01 * NW, (w01 + 1) * NW)
                        rec = recp.tile([P, HEADS], F32, name=f"rec{w01}", tag=f"rec{w01}")
                        for half in range(2):
                            av = avps.tile([P, 8, VBLK], F32, name=f"av{w01}{half}",
                                           tag=f"av{w01}{half}")
                            for hh in range(8):
                                hglob = half * 8 + hh
                                hi, rg = hglob // 4, hglob % 4
                                nc.tensor.matmul(
                                    av[rows, hh, :], pts[rg][rows, hi, :],
                                    v_aug[rows, wp2, hglob * VBLK:(hglob + 1) * VBLK],
                                    start=True, stop=True,
                                    tile_position=(w01 * NW, w01 * NW))
                            nc.vector.reciprocal(rec[rows, half * 8:(half + 1) * 8],
                                                 av[rows, :, HD])
                            rb = rec[rows, half * 8:(half + 1) * 8] \
                                .rearrange("p (a b) -> p a b", b=1).to_broadcast((NW, 8, HD))
                            dst = att[rows, wp2, half * 256:(half + 1) * 256] \
                                .rearrange("p (a b) -> p a b", b=HD)
                            nc.vector.tensor_mul(dst, av[rows, :, 0:HD], rb)

            # attn transpose back (+ inverse shift permute), proj + residual
            if _STAGE < 5:
                continue
            aT_B = hTp.tile([P, KC, L], BF16, name="hT")
            transpose_to(aT_B, att)
            if shift:
                aT = hTp.tile([P, KC, L], BF16, name="hT")
                permute(aT, aT_B, False)
            else:
                aT = aT_B
            wp_sb = wpp.tile([P, KC, C], BF16, name="wp")
            nc.sync.dma_start(wp_sb[:], dr['wp'][i].rearrange("(k p) m -> p k m", p=P))
            if has_bias:
                pbr_sb = bp.tile([1, C], BF16, name="pbr", tag="pbr")
                nc.sync.dma_start(pbr_sb[:], dr['pbr'][i])
            with tc.tile_pool(name="mmps2", bufs=2, space="PSUM") as mmps:
                for tt in range(NT):
                    ps = mmps.tile([P, C], F32, name="mm")
                    if has_bias:
                        nc.tensor.matmul(ps[:], ones_col[:], pbr_sb[:],
                                         start=True, stop=False, skip_group_check=True)
                    for kk in range(KC):
                        nc.tensor.matmul(ps[:], aT[:, kk, tt * P:(tt + 1) * P],
                                         wp_sb[:, kk, :],
                                         start=(kk == 0 and not has_bias),
                                         stop=(kk == KC - 1), skip_group_check=True)
                    nc.vector.tensor_add(x[:, tt], ps[:], x[:, tt])

            # LN2 + h2T
            if _STAGE < 6:
                continue
            h2 = hp.tile([P, NT, C], BF16, name="h")
            layernorm(h2, x)
            h2T = hTp.tile([P, KC, L], BF16, name="hT")
            transpose_to(h2T, h2)

            # MLP
            f1b_sb = bp.tile([P, FH // P], F32, name="f1b", tag="f1b")
            nc.sync.dma_start(f1b_sb[:], dr['f1b'][i])
            if has_bias:
                f2br_sb = bp.tile([1, C], BF16, name="f2br", tag="f2br")
                nc.sync.dma_start(f2br_sb[:], dr['f2br'][i])
            with tc.tile_pool(name="mmps3", bufs=2, space="PSUM") as mmps, \
                 tc.tile_pool(name="fc2ps", bufs=1, space="PSUM") as fc2ps:
                for tc2 in range(2):
                    pso = [fc2ps.tile([P, C], F32, name=f"fc2_{j}", tag=f"fc2_{j}") for j in range(4)]
                    if has_bias:
                        for j in range(4):
                            nc.tensor.matmul(pso[j][:], ones_col[:], f2br_sb[:],
                                             start=True, stop=False, skip_group_check=True)
                    for ho in range(FH // P):
                        f1c = f1p.tile([P, KC, P], BF16, name="f1c")
                        nc.sync.dma_start(f1c[:], dr['f1'][i][:, ho * P:(ho + 1) * P]
                                          .rearrange("(k p) m -> p k m", p=P))
                        f2c = f2p.tile([P, C], BF16, name="f2c")
                        nc.sync.dma_start(f2c[:], dr['f2'][i][ho * P:(ho + 1) * P, :])
                        ps1 = mmps.tile([P, C], F32, name="mm")
                        for kk in range(KC):
                            nc.tensor.matmul(ps1[:], f1c[:, kk, :],
                                             h2T[:, kk, tc2 * 512:(tc2 + 1) * 512],
                                             start=(kk == 0), stop=(kk == KC - 1))
                        g = gp.tile([P, C], BF16, name="g")
                        nc.scalar.activation(g[:], ps1[:], AF.Gelu, bias=f1b_sb[:, ho:ho + 1])
                        for j in range(4):
                            nc.tensor.matmul(pso[j][:], g[:, j * P:(j + 1) * P], f2c[:],
                                             start=(ho == 0 and not has_bias),
                                             stop=(ho == FH // P - 1),
                                             skip_group_check=True)
                    for j in range(4):
                        tt = tc2 * 4 + j
                        nc.vector.tensor_add(x[:, tt], pso[j][:], x[:, tt])

        # ---------- final LN + gelu + pred ----------
        hf = hp.tile([P, NT, C], BF16, name="h")
        layernorm(hf, x)
        nfw_sb = bp.tile([P, KC], F32, name="nfw", tag="nfw")
        nc.sync.dma_start(nfw_sb[:], dr['nfw'])
        nfb_sb = bp.tile([P, KC], F32, name="nfb", tag="nfb")
        nc.sync.dma_start(nfb_sb[:], dr['nfb'])
        pwb_sb = bp.tile([P, N_E // P], F32, name="pwb", tag="pwb", bufs=1)
        nc.sync.dma_start(pwb_sb[:], dr['pwb'])
        gT_pre = hTp.tile([P, KC, L], BF16, name="hT")
        transpose_to(gT_pre, hf)
        gT = hTp.tile([P, KC, L], BF16, name="hT")
        for ct in range(KC):
            nc.scalar.activation(gT[:, ct], gT_pre[:, ct], AF.Gelu,
                                 bias=nfb_sb[:, ct:ct + 1], scale=nfw_sb[:, ct:ct + 1])
        with tc.tile_pool(name="mmpsf", bufs=2, space="PSUM") as mmps:
            for no in range(N_E // P):
                pwc = pwp.tile([P, KC, P], BF16, name="pwc")
                nc.sync.dma_start(pwc[:], dr['pw'][:, no * P:(no + 1) * P]
                                  .rearrange("(k p) m -> p k m", p=P))
                ps = mmps.tile([P, L], F32, name="mm")
                for kk in range(KC):
                    for tc2 in range(2):
                        nc.tensor.matmul(ps[:, tc2 * 512:(tc2 + 1) * 512], pwc[:, kk, :],
                                         gT[:, kk, tc2 * 512:(tc2 + 1) * 512],
                                         start=(kk == 0), stop=(kk == KC - 1),
                                         skip_group_check=True)
                osb = outp.tile([P, L], BF16, name="osb")
                if no % 2 == 0:
                    nc.scalar.activation(osb[:], ps[:], AF.Identity,
                                         bias=pwb_sb[:, no:no + 1])
                else:
                    nc.vector.tensor_scalar_add(osb[:], ps[:], pwb_sb[:, no:no + 1])
                nc.sync.dma_start(outT[no * P:(no + 1) * P, :], osb[:])

    nc.compile()
    _BUILD_CACHE[key] = nc
    return nc


LAST_RESULTS = None


def kernel(**inputs):
    global LAST_RESULTS
    from concourse import bass_utils
    sh, xts = _prepare(inputs)
    nc = _build(_HAS_BIAS)
    in_maps = []
    for c in range(_NCORES):
        m = dict(sh)
        m['xT'] = xts[c % B]
        in_maps.append(m)
    trace = os.environ.get("BT_TRACE", "0") == "1"
    if trace:
        try:
            import antenv.axon_hooks  # noqa: F401
        except ImportError:
            trace = False
    res = bass_utils.run_bass_kernel_spmd(nc, in_maps, core_ids=list(range(_NCORES)),
                                          trace=trace)
    LAST_RESULTS = res
    outs = []
    for c in range(B):
        oT = np.asarray(res.results[c % _NCORES]['outT'], dtype=np.float32)  # [N_E, L]
        o = oT.T[_WM_INV]                      # [L, N_E] raster order
        outs.append(o)
    return np.stack(outs).astype(np.float32)


# revision 11
# speedup vs baseline: 1.4656x; 1.4656x over previous
import os
import numpy as np

# ---- problem constants (hardcoded; kernel.py must be self-contained) ----
IMG, WS, SHIFT = 32, 8, 4
C, HEADS, DEPTH = 512, 16, 24
E_DIM, N_E, B = 256, 8192, 8
L = IMG * IMG            # 1024
NW = WS * WS             # 64 tokens per window
HD = C // HEADS          # 32
NWIN = (IMG // WS) ** 2  # 16
FH = 4 * C               # 2048
P = 128
VBLK = HD + 2            # 34 (32 vals + softmax-denominator col + pad)
VW = HEADS * VBLK        # 544
NT = L // P              # 8 token tiles
KC = C // P              # 4 k-tiles over C
KE = E_DIM // P          # 2 k-tiles over E_DIM
SCALE = HD ** -0.5

_DEPTH = int(os.environ.get("BT_DEPTH", DEPTH))
_NCORES = int(os.environ.get("BT_NCORES", 8))
_STAGE = int(os.environ.get("BT_STAGE", 99))  # debug bisect ladder


# ---- host-side helpers (mirror reference.py) ----
def _rel_index():
    coords = np.stack(np.meshgrid(np.arange(WS), np.arange(WS), indexing='ij'))
    cf = coords.reshape(2, -1)
    rel = (cf[:, :, None] - cf[:, None, :]).transpose(1, 2, 0)
    rel[:, :, 0] += WS - 1
    rel[:, :, 1] += WS - 1
    rel[:, :, 0] *= 2 * WS - 1
    return rel.sum(-1)  # [NW, NW] int


def _shift_mask():
    img = np.zeros((IMG, IMG), np.float32)
    cnt = 0
    sl = (slice(0, -WS), slice(-WS, -SHIFT), slice(-SHIFT, None))
    for hs in sl:
        for ws_ in sl:
            img[hs, ws_] = cnt
            cnt += 1
    win = img.reshape(IMG // WS, WS, IMG // WS, WS).transpose(0, 2, 1, 3).reshape(-1, NW)
    diff = win[:, None, :] - win[:, :, None]
    return np.where(diff != 0, -100.0, 0.0).astype(np.float32)  # [NWIN, NW, NW]


def _win_perm():
    """raster token index -> window-major position; perm[t_raster] = t_dev"""
    t = np.arange(L).reshape(IMG, IMG)
    wm = t.reshape(IMG // WS, WS, IMG // WS, WS).transpose(0, 2, 1, 3).reshape(-1)
    inv = np.empty(L, np.int64)
    inv[wm] = np.arange(L)
    return wm, inv  # wm: dev->raster, inv: raster->dev


_WM, _WM_INV = _win_perm()
_REL = _rel_index()
_MASK = _shift_mask()

_HAS_BIAS = False  # set by _prepare; device build skips rank-1 bias matmuls if False


def _prepare(inputs):
    import ml_dtypes
    global _HAS_BIAS
    BF = ml_dtypes.bfloat16
    f32 = lambda a: np.ascontiguousarray(a, dtype=np.float32)
    bf = lambda a: np.ascontiguousarray(np.asarray(a, np.float32), dtype=BF)
    x = np.asarray(inputs['x'], np.float32)          # [B, L, E]
    dec_w = np.asarray(inputs['dec_w'], np.float32)  # [C, E]
    dec_b = np.asarray(inputs['dec_b'], np.float32)
    pos = np.asarray(inputs['pos_embed'], np.float32)[0]  # [L, C]
    n1w = np.asarray(inputs['n1w'], np.float32)
    n1b = np.asarray(inputs['n1b'], np.float32)
    qkv_w = np.asarray(inputs['qkv_w'], np.float32)
    qkv_b = np.asarray(inputs['qkv_b'], np.float32)
    proj_w = np.asarray(inputs['proj_w'], np.float32)
    proj_b = np.asarray(inputs['proj_b'], np.float32)
    rel_bias = np.asarray(inputs['rel_bias'], np.float32)
    n2w = np.asarray(inputs['n2w'], np.float32)
    n2b = np.asarray(inputs['n2b'], np.float32)
    fc1_w = np.asarray(inputs['fc1_w'], np.float32)
    fc1_b = np.asarray(inputs['fc1_b'], np.float32)
    fc2_w = np.asarray(inputs['fc2_w'], np.float32)
    fc2_b = np.asarray(inputs['fc2_b'], np.float32)
    normf_w = np.asarray(inputs['normf_w'], np.float32)
    normf_b = np.asarray(inputs['normf_b'], np.float32)
    pred_w = np.asarray(inputs['pred_w'], np.float32)
    pred_b = np.asarray(inputs['pred_b'], np.float32)

    D = _DEPTH
    sh = {}
    sh['decw'] = f32(dec_w.T)                                    # [E, C]
    sh['posb'] = f32((pos + dec_b[None, :])[_WM])                # [L, C]

    wqk = np.empty((D, C, 2 * C), BF)
    bqk = np.zeros((D, P, 8), np.float32)
    wvp = np.zeros((D, C, VW), np.float32)
    vb = np.zeros((D, P, VW), np.float32)
    ab = np.empty((D, NT, P, 4, 4, NW), np.float32)  # (wp2, 2w*tk, rg, hi, tq)
    wp_ = np.empty((D, C, C), BF)
    pbr = np.empty((D, 1, C), BF)
    f1 = np.empty((D, C, FH), BF)
    f1b = np.empty((D, P, FH // P), np.float32)
    f2 = np.empty((D, FH, C), BF)
    f2br = np.empty((D, 1, C), BF)

    for i in range(D):
        Wm = qkv_w[i] * n1w[i][None, :]           # [3C, C]
        bm = qkv_w[i] @ n1b[i] + qkv_b[i]         # [3C]
        Wm = Wm.copy()
        bm = bm.copy()
        Wm[:C] *= SCALE
        bm[:C] *= SCALE
        wqk[i] = Wm[:2 * C].T.astype(BF)          # [C, 2C]
        bqk[i] = bm[:2 * C].reshape(8, P).T       # bias for out-channel tile mo at [:, mo]
        # v with padded 34-blocks; ones column + bias via vb (DVE add on copy-out)
        for h in range(HEADS):
            wvp[i][:, h * VBLK:h * VBLK + HD] = Wm[2 * C + h * HD:2 * C + (h + 1) * HD].T
            vb[i][:, h * VBLK:h * VBLK + HD] = bm[2 * C + h * HD:2 * C + (h + 1) * HD][None, :]
            vb[i][:, h * VBLK + HD] = 1.0
        # attention additive bias [tk, tq] per (win, head)
        bias = rel_bias[i][_REL]                  # [tq, tk, HEADS]
        shift = (i % 2) == 1
        for w in range(NWIN):
            for h in range(HEADS):
                a = bias[:, :, h].T               # [tk, tq]
                if shift:
                    a = a + _MASK[w].T
                wp2, w01 = w // 2, w % 2
                rg, hi = h % 4, h // 4
                ab[i, wp2, w01 * NW:(w01 + 1) * NW, rg, hi, :] = a
        wp_[i] = proj_w[i].T.astype(BF)
        pbr[i, 0] = proj_b[i].astype(BF)
        F1m = (fc1_w[i] * n2w[i][None, :]).T      # [C, FH]
        f1[i] = F1m.astype(BF)
        f1b[i] = (fc1_w[i] @ n2b[i] + fc1_b[i]).reshape(FH // P, P).T
        f2[i] = fc2_w[i].T.astype(BF)
        f2br[i, 0] = fc2_b[i].astype(BF)

    _HAS_BIAS = bool(np.any(proj_b != 0) or np.any(fc2_b != 0))

    sh['wqk'] = wqk
    sh['bqk'] = bqk
    sh['wvp'] = bf(wvp)
    sh['vb'] = vb
    sh['ab'] = np.ascontiguousarray(ab.reshape(D, NT, P, 16 * NW).astype(BF))
    sh['wp'] = wp_
    sh['pbr'] = pbr
    sh['f1'] = f1
    sh['f1b'] = f1b
    sh['f2'] = f2
    sh['f2br'] = f2br
    sh['nfw'] = f32(normf_w.reshape(KC, P).T)     # [P, KC]
    sh['nfb'] = f32(normf_b.reshape(KC, P).T)
    sh['pw'] = bf(pred_w.T)                       # [C, N_E]
    sh['pwb'] = f32(pred_b.reshape(N_E // P, P).T)  # [P, 64]
    # per-core xT in device token order: [E, L]
    xts = [np.ascontiguousarray(x[c][_WM].T) for c in range(B)]
    return sh, xts


# ---- device program ----
_BUILD_CACHE = {}


def _build(has_bias):
    key = (_DEPTH, has_bias)
    if key in _BUILD_CACHE:
        return _BUILD_CACHE[key]
    import concourse.bass as bass
    import concourse.mybir as mybir
    import concourse.tile as tile
    from concourse import bacc
    from concourse.masks import make_identity
    from contextlib import ExitStack

    F32 = mybir.dt.float32
    F32R = mybir.dt.float32r
    BF16 = mybir.dt.bfloat16
    AF = mybir.ActivationFunctionType
    ALU = mybir.AluOpType
    AX = mybir.AxisListType
    D = _DEPTH

    nc = bacc.Bacc("TRN2", target_bir_lowering=False, debug=False, num_devices=_NCORES)

    dr = {}
    def din(name, shape, dt):
        dr[name] = nc.dram_tensor(name, list(shape), dt, kind="ExternalInput").ap()
    din('xT', (E_DIM, L), F32R)
    din('decw', (E_DIM, C), F32R)
    din('posb', (L, C), F32)
    din('wqk', (D, C, 2 * C), BF16)
    din('bqk', (D, P, 8), F32)
    din('wvp', (D, C, VW), BF16)
    din('vb', (D, P, VW), F32)
    din('ab', (D, NT, P, 16 * NW), BF16)
    din('wp', (D, C, C), BF16)
    din('pbr', (D, 1, C), BF16)
    din('f1', (D, C, FH), BF16)
    din('f1b', (D, P, FH // P), F32)
    din('f2', (D, FH, C), BF16)
    din('f2br', (D, 1, C), BF16)
    din('nfw', (P, KC), F32)
    din('nfb', (P, KC), F32)
    din('pw', (C, N_E), BF16)
    din('pwb', (P, N_E // P), F32)
    outT = nc.dram_tensor("outT", [N_E, L], BF16, kind="ExternalOutput").ap()

    with tile.TileContext(nc) as tc, ExitStack() as ES:
        # ---------- persistent SBUF pools ----------
        cst = ES.enter_context(tc.tile_pool(name="cst", bufs=1))
        ident_f = cst.tile([P, P], F32)
        make_identity(nc, ident_f)
        ident_b = cst.tile([P, P], BF16)
        nc.scalar.copy(ident_b[:], ident_f[:])
        ones_col = cst.tile([1, P], BF16)
        nc.vector.memset(ones_col[:], 1.0)

        xp = ES.enter_context(tc.tile_pool(name="xp", bufs=1))
        hp = ES.enter_context(tc.tile_pool(name="hp", bufs=2))
        hTp = ES.enter_context(tc.tile_pool(name="hTp", bufs=2))
        qkp = ES.enter_context(tc.tile_pool(name="qkp", bufs=1))
        vp = ES.enter_context(tc.tile_pool(name="vp", bufs=1))
        attp = ES.enter_context(tc.tile_pool(name="attp", bufs=1))
        ppool = ES.enter_context(tc.tile_pool(name="ppool", bufs=2))
        abp = ES.enter_context(tc.tile_pool(name="abp", bufs=3))
        stp = ES.enter_context(tc.tile_pool(name="stp", bufs=2))
        recp = ES.enter_context(tc.tile_pool(name="recp", bufs=2))
        wqkp = ES.enter_context(tc.tile_pool(name="wqkp", bufs=2))
        wvpp = ES.enter_context(tc.tile_pool(name="wvpp", bufs=2))
        wpp = ES.enter_context(tc.tile_pool(name="wpp", bufs=2))
        f1p = ES.enter_context(tc.tile_pool(name="f1p", bufs=4))
        f2p = ES.enter_context(tc.tile_pool(name="f2p", bufs=4))
        gp = ES.enter_context(tc.tile_pool(name="gp", bufs=3))
        bp = ES.enter_context(tc.tile_pool(name="bp", bufs=2))
        outp = ES.enter_context(tc.tile_pool(name="outp", bufs=3))
        pwp = ES.enter_context(tc.tile_pool(name="pwp", bufs=4))

        x = xp.tile([P, NT, C], F32)

        # ---------- dec ----------
        with tc.tile_pool(name="decp", bufs=1) as decp, \
             tc.tile_pool(name="dps", bufs=2, space="PSUM") as dps:
            xT_sb = decp.tile([P, KE, L], F32R)
            nc.sync.dma_start(xT_sb[:], dr['xT'].rearrange("(k p) t -> p k t", p=P))
            decw_sb = decp.tile([P, KE, C], F32R)
            nc.sync.dma_start(decw_sb[:], dr['decw'].rearrange("(k p) c -> p k c", p=P))
            for tt in range(NT):
                pos_t = decp.tile([P, C], F32, name="pos_t", tag="pos", bufs=2)
                nc.sync.dma_start(pos_t[:], dr['posb'][tt * P:(tt + 1) * P, :])
                ps = dps.tile([P, C], F32)
                for kk in range(KE):
                    nc.tensor.matmul(ps[:], xT_sb[:, kk, tt * P:(tt + 1) * P],
                                     decw_sb[:, kk, :], start=(kk == 0), stop=(kk == KE - 1))
                nc.vector.tensor_add(x[:, tt], ps[:], pos_t[:])

        # ---------- layer-norm helper: bn_stats path, bf16 out ----------
        def layernorm(dst, src):
            for tt in range(NT):
                st6 = stp.tile([P, 6], F32, name="st6", tag=f"st6{tt % 2}")
                nc.vector.bn_stats(st6[:], src[:, tt])
                mv = stp.tile([P, 2], F32, name="mv", tag=f"mv{tt % 2}")
                nc.vector.bn_aggr(mv[:], st6[:])
                veps = stp.tile([P, 1], F32, name="veps", tag=f"veps{tt % 2}")
                nc.vector.tensor_scalar(veps[:], mv[:, 1:2], 1.0, 1e-5,
                                        ALU.mult, ALU.add)
                rstd = stp.tile([P, 1], F32, name="rstd", tag=f"rstd{tt % 2}")
                nc.scalar.activation(rstd[:], veps[:], AF.Sqrt)
                nc.vector.reciprocal(rstd[:], rstd[:])
                nb = stp.tile([P, 1], F32, name="nb", tag=f"nb{tt % 2}")
                nc.vector.tensor_scalar(nb[:], mv[:, 0:1], rstd[:, 0:1], -1.0,
                                        ALU.mult, ALU.mult)
                nc.scalar.activation(dst[:, tt], src[:, tt], AF.Identity,
                                     bias=nb[:], scale=rstd[:])

        # token-major [P, NT, C] bf16 -> C-major [P, NT, KC, P] bf16 via DMA
        # transpose: hT[:, tt, ct, :] = src[:, tt, ct*P:(ct+1)*P].T.
        # dst per instruction is [P, KC, P], per-partition contiguous (1KB).
        def transpose_to(hT, src):
            for tt in range(NT):
                nc.sync.dma_start_transpose(hT[:, tt], src[:, tt, :])

        # shift permute between A-order and B-order on [P, NT, KC, P] tiles.
        # token t = a*256 + b*64 + i*8 + j; tile dims: (a bh) over NT with
        # b = bh*2 + bl, and (bl i j) over the 128-token chunk.
        def permute(dstT, srcT, fwd):
            G = IMG // WS  # 4
            sv = srcT[:].rearrange("p (a bh) k (bl i j) -> p a bh k bl i j",
                                   a=G, bh=2, bl=2, i=WS, j=WS)
            dv = dstT[:].rearrange("p (a bh) k (bl i j) -> p a bh k bl i j",
                                   a=G, bh=2, bl=2, i=WS, j=WS)
            FULL = slice(0, 2)
            for qa in range(2):
                for qb in range(2):
                    di = slice(0, 4) if qa == 0 else slice(4, 8)
                    si = slice(4, 8) if qa == 0 else slice(0, 4)
                    dj = slice(0, 4) if qb == 0 else slice(4, 8)
                    sj = slice(4, 8) if qb == 0 else slice(0, 4)
                    for a in range(G):
                        sa = (a + qa) % G
                        if qb == 0:
                            # sb == b: full (bh, bl) block copy
                            moves = [((FULL, FULL), (FULL, FULL))]
                        else:
                            # sb = b + 1 mod 4 decomposed on (bh, bl)
                            moves = [((FULL, slice(0, 1)), (FULL, slice(1, 2))),
                                     ((slice(0, 1), slice(1, 2)), (slice(1, 2), slice(0, 1))),
                                     ((slice(1, 2), slice(1, 2)), (slice(0, 1), slice(0, 1)))]
                        for (dbh, dbl), (sbh, sbl) in moves:
                            for ct in range(KC):
                                eng = (nc.gpsimd, nc.vector)[(a + ct) % 2]
                                if fwd:
                                    eng.tensor_copy(dv[:, a, dbh, ct, dbl, di, dj],
                                                    sv[:, sa, sbh, ct, sbl, si, sj])
                                else:
                                    eng.tensor_copy(dv[:, sa, sbh, ct, sbl, si, sj],
                                                    sv[:, a, dbh, ct, dbl, di, dj])

        # ---------- layers ----------
        for i in range(D):
            if _STAGE < 1:
                break
            shift = (i % 2) == 1
            # LN1 -> h (bf16)
            h = hp.tile([P, NT, C], BF16, name="h")
            layernorm(h, x)
            hT_A = hTp.tile([P, NT, KC, P], BF16, name="hT")
            transpose_to(hT_A, h)
            if shift:
                hT = hTp.tile([P, NT, KC, P], BF16, name="hT")
                permute(hT, hT_A, True)
            else:
                hT = hT_A

            # qk + v
            if _STAGE < 2:
                continue
            wqk_sb = wqkp.tile([P, KC, 2 * C], BF16, name="wqk")
            nc.sync.dma_start(wqk_sb[:], dr['wqk'][i].rearrange("(k p) m -> p k m", p=P))
            bqk_sb = bp.tile([P, 8], F32, name="bqk", tag="bqk")
            nc.sync.dma_start(bqk_sb[:], dr['bqk'][i])
            qkT = qkp.tile([P, 8, L], BF16, name="qkT")
            wvp_sb = wvpp.tile([P, KC, VW], BF16, name="wvp")
            nc.sync.dma_start(wvp_sb[:], dr['wvp'][i].rearrange("(k p) m -> p k m", p=P))
            vb_sb = bp.tile([P, VW], F32, name="vb", tag="vb")
            nc.sync.dma_start(vb_sb[:], dr['vb'][i])
            v_aug = vp.tile([P, NT, VW], BF16, name="vaug")
            with tc.tile_pool(name="mmps1", bufs=2, space="PSUM") as mmps, \
                 tc.tile_pool(name="vps", bufs=2, space="PSUM") as vps:
                for mo in range(8):
                    ps = mmps.tile([P, L], F32, name="mm")
                    for kk in range(KC):
                        for tc2 in range(2):
                            nc.tensor.matmul(ps[:, tc2 * 512:(tc2 + 1) * 512],
                                             wqk_sb[:, kk, mo * P:(mo + 1) * P],
                                             hT[:, tc2 * 4:(tc2 + 1) * 4, kk, :],
                                             start=(kk == 0), stop=(kk == KC - 1),
                                             skip_group_check=True)
                    nc.scalar.activation(qkT[:, mo], ps[:], AF.Identity,
                                         bias=bqk_sb[:, mo:mo + 1])
                for tt in range(NT if _STAGE >= 3 else 0):
                    psv = vps.tile([P, VW], F32, name="vps")
                    for kk in range(KC):
                        nc.tensor.matmul(psv[:, 0:512], hT[:, tt, kk, :],
                                         wvp_sb[:, kk, 0:512], start=(kk == 0),
                                         stop=(kk == KC - 1), skip_group_check=True)
                        nc.tensor.matmul(psv[:, 512:VW], hT[:, tt, kk, :],
                                         wvp_sb[:, kk, 512:VW], start=(kk == 0),
                                         stop=(kk == KC - 1), skip_group_check=True)
                    nc.vector.tensor_add(v_aug[:, tt], psv[:], vb_sb[:])

            if _STAGE < 4:
                continue
            # attention: S + softmax + AV
            att = attp.tile([P, NT, C], BF16, name="att")
            with tc.tile_pool(name="sps", bufs=1, space="PSUM") as sps, \
                 tc.tile_pool(name="avps", bufs=1, space="PSUM") as avps:
                for wp2 in range(NT):
                    abt = abp.tile([P, 16 * NW], BF16, name=f"ab{wp2 % 3}")
                    nc.sync.dma_start(abt[:], dr['ab'][i, wp2])
                    pts = []
                    for rg in range(4):
                        sp = sps.tile([P, 4, NW], F32, name=f"s{rg}", tag=f"s{rg}")
                        spf = sp[:].rearrange("p a b -> p (a b)")
                        nc.tensor.matmul(spf, ident_b[:],
                                         abt[:, rg * 256:(rg + 1) * 256],
                                         start=True, stop=False, skip_group_check=True)
                        for hi in range(4):
                            for w01 in range(2):
                                qs = qkT[rg * HD:(rg + 1) * HD, hi,
                                         (wp2 * 2 + w01) * NW:(wp2 * 2 + w01 + 1) * NW]
                                ks = qkT[rg * HD:(rg + 1) * HD, 4 + hi,
                                         (wp2 * 2 + w01) * NW:(wp2 * 2 + w01 + 1) * NW]
                                nc.tensor.matmul(sp[w01 * NW:(w01 + 1) * NW, hi, :],
                                                 ks, qs, start=False,
                                                 stop=(hi == 3 and w01 == 1),
                                                 tile_position=(rg * HD, w01 * NW),
                                                 skip_group_check=True)
                        pt = ppool.tile([P, 4, NW], BF16, name=f"p{rg}", tag=f"p{rg}")
                        nc.scalar.activation(pt[:].rearrange("p a b -> p (a b)"),
                                             spf, AF.Exp)
                        pts.append(pt)
                    for w01 in range(2):
                        rows = slice(w01 * NW, (w01 + 1) * NW)
                        rec = recp.tile([P, HEADS], F32, name=f"rec{w01}", tag=f"rec{w01}")
                        for half in range(2):
                            av = avps.tile([P, 8, VBLK], F32, name=f"av{w01}{half}",
                                           tag=f"av{w01}{half}")
                            for hh in range(8):
                                hglob = half * 8 + hh
                                hi, rg = hglob // 4, hglob % 4
                                nc.tensor.matmul(
                                    av[rows, hh, :], pts[rg][rows, hi, :],
                                    v_aug[rows, wp2, hglob * VBLK:(hglob + 1) * VBLK],
                                    start=True, stop=True,
                                    tile_position=(w01 * NW, w01 * NW))
                            nc.vector.reciprocal(rec[rows, half * 8:(half + 1) * 8],
                                                 av[rows, :, HD])
                            rb = rec[rows, half * 8:(half + 1) * 8] \
                                .rearrange("p (a b) -> p a b", b=1).to_broadcast((NW, 8, HD))
                            dst = att[rows, wp2, half * 256:(half + 1) * 256] \
                                .rearrange("p (a b) -> p a b", b=HD)
                            nc.vector.tensor_mul(dst, av[rows, :, 0:HD], rb)

            # attn transpose back (+ inverse shift permute), proj + residual
            if _STAGE < 5:
                continue
            aT_B = hTp.tile([P, NT, KC, P], BF16, name="hT")
            transpose_to(aT_B, att)
            if shift:
                aT = hTp.tile([P, NT, KC, P], BF16, name="hT")
                permute(aT, aT_B, False)
            else:
                aT = aT_B
            wp_sb = wpp.tile([P, KC, C], BF16, name="wp")
            nc.sync.dma_start(wp_sb[:], dr['wp'][i].rearrange("(k p) m -> p k m", p=P))
            if has_bias:
                pbr_sb = bp.tile([1, C], BF16, name="pbr", tag="pbr")
                nc.sync.dma_start(pbr_sb[:], dr['pbr'][i])
            with tc.tile_pool(name="mmps2", bufs=2, space="PSUM") as mmps:
                for tt in range(NT):
                    ps = mmps.tile([P, C], F32, name="mm")
                    if has_bias:
                        nc.tensor.matmul(ps[:], ones_col[:], pbr_sb[:],
                                         start=True, stop=False, skip_group_check=True)
                    for kk in range(KC):
                        nc.tensor.matmul(ps[:], aT[:, tt, kk, :],
                                         wp_sb[:, kk, :],
                                         start=(kk == 0 and not has_bias),
                                         stop=(kk == KC - 1), skip_group_check=True)
                    nc.vector.tensor_add(x[:, tt], ps[:], x[:, tt])

            # LN2 + h2T
            if _STAGE < 6:
                continue
            h2 = hp.tile([P, NT, C], BF16, name="h")
            layernorm(h2, x)
            h2T = hTp.tile([P, NT, KC, P], BF16, name="hT")
            transpose_to(h2T, h2)

            # MLP
            f1b_sb = bp.tile([P, FH // P], F32, name="f1b", tag="f1b")
            nc.sync.dma_start(f1b_sb[:], dr['f1b'][i])
            if has_bias:
                f2br_sb = bp.tile([1, C], BF16, name="f2br", tag="f2br")
                nc.sync.dma_start(f2br_sb[:], dr['f2br'][i])
            with tc.tile_pool(name="mmps3", bufs=2, space="PSUM") as mmps, \
                 tc.tile_pool(name="fc2ps", bufs=1, space="PSUM") as fc2ps:
                for tc2 in range(2):
                    pso = [fc2ps.tile([P, C], F32, name=f"fc2_{j}", tag=f"fc2_{j}") for j in range(4)]
                    if has_bias:
                        for j in range(4):
                            nc.tensor.matmul(pso[j][:], ones_col[:], f2br_sb[:],
                                             start=True, stop=False, skip_group_check=True)
                    for ho4 in range(4):
                        f1c = f1p.tile([P, KC, 4, P], BF16, name="f1c")
                        nc.sync.dma_start(f1c[:], dr['f1'][i][:, ho4 * 512:(ho4 + 1) * 512]
                                            .rearrange("(k p) (h m) -> p k h m", p=P, m=P))
                        f2c = f2p.tile([P, 4, C], BF16, name="f2c")
                        nc.sync.dma_start(f2c[:], dr['f2'][i][ho4 * 512:(ho4 + 1) * 512, :]
                                            .rearrange("(h p) m -> p h m", p=P))
                        for hs in range(4):
                            ho = ho4 * 4 + hs
                            ps1 = mmps.tile([P, C], F32, name="mm")
                            for kk in range(KC):
                                nc.tensor.matmul(ps1[:], f1c[:, kk, hs, :],
                                                 h2T[:, tc2 * 4:(tc2 + 1) * 4, kk, :],
                                                 start=(kk == 0), stop=(kk == KC - 1))
                            g = gp.tile([P, C], BF16, name="g")
                            nc.scalar.activation(g[:], ps1[:], AF.Gelu, bias=f1b_sb[:, ho:ho + 1])
                            for j in range(4):
                                nc.tensor.matmul(pso[j][:], g[:, j * P:(j + 1) * P], f2c[:, hs, :],
                                                 start=(ho == 0 and not has_bias),
                                                 stop=(ho == FH // P - 1),
                                                 skip_group_check=True)
                    for j in range(4):
                        tt = tc2 * 4 + j
                        nc.vector.tensor_add(x[:, tt], pso[j][:], x[:, tt])

        # ---------- final LN + gelu + pred ----------
        hf = hp.tile([P, NT, C], BF16, name="h")
        layernorm(hf, x)
        nfw_sb = bp.tile([P, KC], F32, name="nfw", tag="nfw")
        nc.sync.dma_start(nfw_sb[:], dr['nfw'])
        nfb_sb = bp.tile([P, KC], F32, name="nfb", tag="nfb")
        nc.sync.dma_start(nfb_sb[:], dr['nfb'])
        pwb_sb = bp.tile([P, N_E // P], F32, name="pwb", tag="pwb", bufs=1)
        nc.sync.dma_start(pwb_sb[:], dr['pwb'])
        gT_pre = hTp.tile([P, NT, KC, P], BF16, name="hT")
        transpose_to(gT_pre, hf)
        gT = hTp.tile([P, NT, KC, P], BF16, name="hT")
        for ct in range(KC):
            nc.scalar.activation(gT[:, :, ct, :], gT_pre[:, :, ct, :], AF.Gelu,
                                 bias=nfb_sb[:, ct:ct + 1], scale=nfw_sb[:, ct:ct + 1])
        with tc.tile_pool(name="mmpsf", bufs=2, space="PSUM") as mmps:
            for no in range(N_E // P):
                pwc = pwp.tile([P, KC, P], BF16, name="pwc")
                nc.sync.dma_start(pwc[:], dr['pw'][:, no * P:(no + 1) * P]
                                  .rearrange("(k p) m -> p k m", p=P))
                ps = mmps.tile([P, L], F32, name="mm")
                for kk in range(KC):
                    for tc2 in range(2):
                        nc.tensor.matmul(ps[:, tc2 * 512:(tc2 + 1) * 512], pwc[:, kk, :],
                                         gT[:, tc2 * 4:(tc2 + 1) * 4, kk, :],
                                         start=(kk == 0), stop=(kk == KC - 1),
                                         skip_group_check=True)
                osb = outp.tile([P, L], BF16, name="osb")
                if no % 2 == 0:
                    nc.scalar.activation(osb[:], ps[:], AF.Identity,
                                         bias=pwb_sb[:, no:no + 1])
                else:
                    nc.vector.tensor_scalar_add(osb[:], ps[:], pwb_sb[:, no:no + 1])
                nc.sync.dma_start(outT[no * P:(no + 1) * P, :], osb[:])

    nc.compile()
    _BUILD_CACHE[key] = nc
    return nc


LAST_RESULTS = None


def kernel(**inputs):
    global LAST_RESULTS
    from concourse import bass_utils
    sh, xts = _prepare(inputs)
    nc = _build(_HAS_BIAS)
    in_maps = []
    for c in range(_NCORES):
        m = dict(sh)
        m['xT'] = xts[c % B]
        in_maps.append(m)
    trace = os.environ.get("BT_TRACE", "0") == "1"
    if trace:
        try:
            import antenv.axon_hooks  # noqa: F401
        except ImportError:
            trace = False
    res = bass_utils.run_bass_kernel_spmd(nc, in_maps, core_ids=list(range(_NCORES)),
                                          trace=trace)
    LAST_RESULTS = res
    outs = []
    for c in range(B):
        oT = np.asarray(res.results[c % _NCORES]['outT'], dtype=np.float32)  # [N_E, L]
        o = oT.T[_WM_INV]                      # [L, N_E] raster order
        outs.append(o)
    return np.stack(outs).astype(np.float32)


# revision 12
# speedup vs baseline: 1.7039x; 1.1626x over previous
import os
import numpy as np

# ---- problem constants (hardcoded; kernel.py must be self-contained) ----
IMG, WS, SHIFT = 32, 8, 4
C, HEADS, DEPTH = 512, 16, 24
E_DIM, N_E, B = 256, 8192, 8
L = IMG * IMG            # 1024
NW = WS * WS             # 64 tokens per window
HD = C // HEADS          # 32
NWIN = (IMG // WS) ** 2  # 16
FH = 4 * C               # 2048
P = 128
VBLK = HD + 2            # 34 (32 vals + softmax-denominator col + pad)
VW = HEADS * VBLK        # 544
NT = L // P              # 8 token tiles
KC = C // P              # 4 k-tiles over C
KE = E_DIM // P          # 2 k-tiles over E_DIM
SCALE = HD ** -0.5

_DEPTH = int(os.environ.get("BT_DEPTH", DEPTH))
_NCORES = int(os.environ.get("BT_NCORES", 8))
_STAGE = int(os.environ.get("BT_STAGE", 99))  # debug bisect ladder


# ---- host-side helpers (mirror reference.py) ----
def _rel_index():
    coords = np.stack(np.meshgrid(np.arange(WS), np.arange(WS), indexing='ij'))
    cf = coords.reshape(2, -1)
    rel = (cf[:, :, None] - cf[:, None, :]).transpose(1, 2, 0)
    rel[:, :, 0] += WS - 1
    rel[:, :, 1] += WS - 1
    rel[:, :, 0] *= 2 * WS - 1
    return rel.sum(-1)  # [NW, NW] int


def _shift_mask():
    img = np.zeros((IMG, IMG), np.float32)
    cnt = 0
    sl = (slice(0, -WS), slice(-WS, -SHIFT), slice(-SHIFT, None))
    for hs in sl:
        for ws_ in sl:
            img[hs, ws_] = cnt
            cnt += 1
    win = img.reshape(IMG // WS, WS, IMG // WS, WS).transpose(0, 2, 1, 3).reshape(-1, NW)
    diff = win[:, None, :] - win[:, :, None]
    return np.where(diff != 0, -100.0, 0.0).astype(np.float32)  # [NWIN, NW, NW]


def _win_perm():
    """raster token index -> window-major position; perm[t_raster] = t_dev"""
    t = np.arange(L).reshape(IMG, IMG)
    wm = t.reshape(IMG // WS, WS, IMG // WS, WS).transpose(0, 2, 1, 3).reshape(-1)
    inv = np.empty(L, np.int64)
    inv[wm] = np.arange(L)
    return wm, inv  # wm: dev->raster, inv: raster->dev


_WM, _WM_INV = _win_perm()
_REL = _rel_index()
_MASK = _shift_mask()

_HAS_BIAS = False  # set by _prepare; device build skips rank-1 bias matmuls if False


def _prepare(inputs):
    import ml_dtypes
    global _HAS_BIAS
    BF = ml_dtypes.bfloat16
    f32 = lambda a: np.ascontiguousarray(a, dtype=np.float32)
    bf = lambda a: np.ascontiguousarray(np.asarray(a, np.float32), dtype=BF)
    x = np.asarray(inputs['x'], np.float32)          # [B, L, E]
    dec_w = np.asarray(inputs['dec_w'], np.float32)  # [C, E]
    dec_b = np.asarray(inputs['dec_b'], np.float32)
    pos = np.asarray(inputs['pos_embed'], np.float32)[0]  # [L, C]
    n1w = np.asarray(inputs['n1w'], np.float32)
    n1b = np.asarray(inputs['n1b'], np.float32)
    qkv_w = np.asarray(inputs['qkv_w'], np.float32)
    qkv_b = np.asarray(inputs['qkv_b'], np.float32)
    proj_w = np.asarray(inputs['proj_w'], np.float32)
    proj_b = np.asarray(inputs['proj_b'], np.float32)
    rel_bias = np.asarray(inputs['rel_bias'], np.float32)
    n2w = np.asarray(inputs['n2w'], np.float32)
    n2b = np.asarray(inputs['n2b'], np.float32)
    fc1_w = np.asarray(inputs['fc1_w'], np.float32)
    fc1_b = np.asarray(inputs['fc1_b'], np.float32)
    fc2_w = np.asarray(inputs['fc2_w'], np.float32)
    fc2_b = np.asarray(inputs['fc2_b'], np.float32)
    normf_w = np.asarray(inputs['normf_w'], np.float32)
    normf_b = np.asarray(inputs['normf_b'], np.float32)
    pred_w = np.asarray(inputs['pred_w'], np.float32)
    pred_b = np.asarray(inputs['pred_b'], np.float32)

    D = _DEPTH
    sh = {}
    sh['decw'] = f32(dec_w.T)                                    # [E, C]
    sh['posb'] = f32((pos + dec_b[None, :])[_WM])                # [L, C]

    wqk = np.empty((D, C, 2 * C), BF)
    bqk = np.zeros((D, P, 8), np.float32)
    wvp = np.zeros((D, C, VW), np.float32)
    vb = np.zeros((D, P, VW), np.float32)
    ab = np.empty((D, NT, P, 4, 4, NW), np.float32)  # (wp2, 2w*tk, rg, hi, tq)
    wp_ = np.empty((D, C, C), BF)
    pbr = np.empty((D, 1, C), BF)
    f1 = np.empty((D, C, FH), BF)
    f1b = np.empty((D, P, FH // P), np.float32)
    f2 = np.empty((D, FH, C), BF)
    f2br = np.empty((D, 1, C), BF)

    for i in range(D):
        Wm = qkv_w[i] * n1w[i][None, :]           # [3C, C]
        bm = qkv_w[i] @ n1b[i] + qkv_b[i]         # [3C]
        Wm = Wm.copy()
        bm = bm.copy()
        Wm[:C] *= SCALE
        bm[:C] *= SCALE
        wqk[i] = Wm[:2 * C].T.astype(BF)          # [C, 2C]
        bqk[i] = bm[:2 * C].reshape(8, P).T       # bias for out-channel tile mo at [:, mo]
        # v with padded 34-blocks; ones column + bias via vb (DVE add on copy-out)
        for h in range(HEADS):
            wvp[i][:, h * VBLK:h * VBLK + HD] = Wm[2 * C + h * HD:2 * C + (h + 1) * HD].T
            vb[i][:, h * VBLK:h * VBLK + HD] = bm[2 * C + h * HD:2 * C + (h + 1) * HD][None, :]
            vb[i][:, h * VBLK + HD] = 1.0
        # attention additive bias [tk, tq] per (win, head)
        bias = rel_bias[i][_REL]                  # [tq, tk, HEADS]
        shift = (i % 2) == 1
        for w in range(NWIN):
            for h in range(HEADS):
                a = bias[:, :, h].T               # [tk, tq]
                if shift:
                    a = a + _MASK[w].T
                wp2, w01 = w // 2, w % 2
                rg, hi = h % 4, h // 4
                ab[i, wp2, w01 * NW:(w01 + 1) * NW, rg, hi, :] = a
        wp_[i] = proj_w[i].T.astype(BF)
        pbr[i, 0] = proj_b[i].astype(BF)
        F1m = (fc1_w[i] * n2w[i][None, :]).T      # [C, FH]
        f1[i] = F1m.astype(BF)
        f1b[i] = (fc1_w[i] @ n2b[i] + fc1_b[i]).reshape(FH // P, P).T
        f2[i] = fc2_w[i].T.astype(BF)
        f2br[i, 0] = fc2_b[i].astype(BF)

    _HAS_BIAS = bool(np.any(proj_b != 0) or np.any(fc2_b != 0))

    sh['wqk'] = wqk
    sh['bqk'] = bqk
    sh['wvp'] = bf(wvp)
    sh['vb'] = vb
    sh['ab'] = np.ascontiguousarray(ab.reshape(D, NT, P, 16 * NW).astype(BF))
    sh['wp'] = wp_
    sh['pbr'] = pbr
    sh['f1'] = f1
    sh['f1b'] = f1b
    sh['f2'] = f2
    sh['f2br'] = f2br
    sh['nfw'] = f32(normf_w.reshape(KC, P).T)     # [P, KC]
    sh['nfb'] = f32(normf_b.reshape(KC, P).T)
    sh['pw'] = bf(pred_w.T)                       # [C, N_E]
    sh['pwb'] = f32(pred_b.reshape(N_E // P, P).T)  # [P, 64]
    # per-core xT in device token order: [E, L]
    xts = [np.ascontiguousarray(x[c][_WM].T) for c in range(B)]
    return sh, xts


# ---- device program ----
_BUILD_CACHE = {}


def _build(has_bias):
    key = (_DEPTH, has_bias)
    if key in _BUILD_CACHE:
        return _BUILD_CACHE[key]
    import concourse.bass as bass
    import concourse.mybir as mybir
    import concourse.tile as tile
    from concourse import bacc
    from concourse.masks import make_identity
    from contextlib import ExitStack

    F32 = mybir.dt.float32
    F32R = mybir.dt.float32r
    BF16 = mybir.dt.bfloat16
    AF = mybir.ActivationFunctionType
    ALU = mybir.AluOpType
    AX = mybir.AxisListType
    D = _DEPTH

    nc = bacc.Bacc("TRN2", target_bir_lowering=False, debug=False, num_devices=_NCORES)

    dr = {}
    def din(name, shape, dt):
        dr[name] = nc.dram_tensor(name, list(shape), dt, kind="ExternalInput").ap()
    din('xT', (E_DIM, L), F32R)
    din('decw', (E_DIM, C), F32R)
    din('posb', (L, C), F32)
    din('wqk', (D, C, 2 * C), BF16)
    din('bqk', (D, P, 8), F32)
    din('wvp', (D, C, VW), BF16)
    din('vb', (D, P, VW), F32)
    din('ab', (D, NT, P, 16 * NW), BF16)
    din('wp', (D, C, C), BF16)
    din('pbr', (D, 1, C), BF16)
    din('f1', (D, C, FH), BF16)
    din('f1b', (D, P, FH // P), F32)
    din('f2', (D, FH, C), BF16)
    din('f2br', (D, 1, C), BF16)
    din('nfw', (P, KC), F32)
    din('nfb', (P, KC), F32)
    din('pw', (C, N_E), BF16)
    din('pwb', (P, N_E // P), F32)
    outT = nc.dram_tensor("outT", [N_E, L], BF16, kind="ExternalOutput").ap()

    with tile.TileContext(nc) as tc, ExitStack() as ES:
        # ---------- persistent SBUF pools ----------
        cst = ES.enter_context(tc.tile_pool(name="cst", bufs=1))
        ident_f = cst.tile([P, P], F32)
        make_identity(nc, ident_f)
        ident_b = cst.tile([P, P], BF16)
        nc.scalar.copy(ident_b[:], ident_f[:])
        ones_col = cst.tile([1, P], BF16)
        nc.vector.memset(ones_col[:], 1.0)

        xp = ES.enter_context(tc.tile_pool(name="xp", bufs=1))
        hp = ES.enter_context(tc.tile_pool(name="hp", bufs=2))
        hTp = ES.enter_context(tc.tile_pool(name="hTp", bufs=2))
        qkp = ES.enter_context(tc.tile_pool(name="qkp", bufs=1))
        vp = ES.enter_context(tc.tile_pool(name="vp", bufs=1))
        attp = ES.enter_context(tc.tile_pool(name="attp", bufs=1))
        ppool = ES.enter_context(tc.tile_pool(name="ppool", bufs=2))
        abp = ES.enter_context(tc.tile_pool(name="abp", bufs=3))
        stp = ES.enter_context(tc.tile_pool(name="stp", bufs=2))
        recp = ES.enter_context(tc.tile_pool(name="recp", bufs=2))
        wqkp = ES.enter_context(tc.tile_pool(name="wqkp", bufs=2))
        wvpp = ES.enter_context(tc.tile_pool(name="wvpp", bufs=2))
        wpp = ES.enter_context(tc.tile_pool(name="wpp", bufs=2))
        f1p = ES.enter_context(tc.tile_pool(name="f1p", bufs=4))
        f2p = ES.enter_context(tc.tile_pool(name="f2p", bufs=4))
        gp = ES.enter_context(tc.tile_pool(name="gp", bufs=3))
        bp = ES.enter_context(tc.tile_pool(name="bp", bufs=2))
        outp = ES.enter_context(tc.tile_pool(name="outp", bufs=3))
        pwp = ES.enter_context(tc.tile_pool(name="pwp", bufs=4))

        x = xp.tile([P, NT, C], F32)

        # ---------- dec ----------
        with tc.tile_pool(name="decp", bufs=1) as decp, \
             tc.tile_pool(name="dps", bufs=2, space="PSUM") as dps:
            xT_sb = decp.tile([P, KE, L], F32R)
            nc.sync.dma_start(xT_sb[:], dr['xT'].rearrange("(k p) t -> p k t", p=P))
            decw_sb = decp.tile([P, KE, C], F32R)
            nc.sync.dma_start(decw_sb[:], dr['decw'].rearrange("(k p) c -> p k c", p=P))
            for tt in range(NT):
                pos_t = decp.tile([P, C], F32, name="pos_t", tag="pos", bufs=2)
                nc.sync.dma_start(pos_t[:], dr['posb'][tt * P:(tt + 1) * P, :])
                ps = dps.tile([P, C], F32)
                for kk in range(KE):
                    nc.tensor.matmul(ps[:], xT_sb[:, kk, tt * P:(tt + 1) * P],
                                     decw_sb[:, kk, :], start=(kk == 0), stop=(kk == KE - 1))
                nc.vector.tensor_add(x[:, tt], ps[:], pos_t[:])

        # ---------- layer-norm helper: bn_stats path, bf16 out ----------
        def layernorm(dst, src):
            with tc.tile_pool(name="warm", bufs=1, space="PSUM") as warmp:
                _layernorm(dst, src, warmp)

        def _layernorm(dst, src, warmp):
            for tt in range(NT):
                st6 = stp.tile([P, 6], F32, name="st6", tag=f"st6{tt % 2}")
                nc.vector.bn_stats(st6[:], src[:, tt])
                wps = warmp.tile([1, 6], F32, name="w")
                nc.tensor.matmul(wps[:], ident_f[0:1, 0:1], st6[0:1, :],
                                 start=True, stop=True, skip_group_check=True)
                mv = stp.tile([P, 2], F32, name="mv", tag=f"mv{tt % 2}")
                nc.vector.bn_aggr(mv[:], st6[:])
                veps = stp.tile([P, 1], F32, name="veps", tag=f"veps{tt % 2}")
                nc.vector.tensor_scalar(veps[:], mv[:, 1:2], 1.0, 1e-5,
                                        ALU.mult, ALU.add)
                rstd = stp.tile([P, 1], F32, name="rstd", tag=f"rstd{tt % 2}")
                nc.scalar.activation(rstd[:], veps[:], AF.Sqrt)
                nc.vector.reciprocal(rstd[:], rstd[:])
                nb = stp.tile([P, 1], F32, name="nb", tag=f"nb{tt % 2}")
                nc.vector.tensor_scalar(nb[:], mv[:, 0:1], rstd[:, 0:1], -1.0,
                                        ALU.mult, ALU.mult)
                nc.scalar.activation(dst[:, tt], src[:, tt], AF.Identity,
                                     bias=nb[:], scale=rstd[:])

        # token-major [P, NT, C] bf16 -> C-major [P, NT, KC, P] bf16 via DMA
        # transpose: hT[:, tt, ct, :] = src[:, tt, ct*P:(ct+1)*P].T.
        # dst per instruction is [P, KC, P], per-partition contiguous (1KB).
        def transpose_to(hT, src):
            for tt in range(NT):
                nc.sync.dma_start_transpose(hT[:, tt], src[:, tt, :])

        # shift permute between A-order and B-order on [P, NT, KC, P] tiles.
        # token t = a*256 + b*64 + i*8 + j; tile dims: (a bh) over NT with
        # b = bh*2 + bl, and (bl i j) over the 128-token chunk.
        def permute(dstT, srcT, fwd):
            G = IMG // WS  # 4
            sv = srcT[:].rearrange("p (a bh) k (bl i j) -> p a bh k bl i j",
                                   a=G, bh=2, bl=2, i=WS, j=WS)
            dv = dstT[:].rearrange("p (a bh) k (bl i j) -> p a bh k bl i j",
                                   a=G, bh=2, bl=2, i=WS, j=WS)
            FULL = slice(0, 2)
            for qa in range(2):
                for qb in range(2):
                    di = slice(0, 4) if qa == 0 else slice(4, 8)
                    si = slice(4, 8) if qa == 0 else slice(0, 4)
                    dj = slice(0, 4) if qb == 0 else slice(4, 8)
                    sj = slice(4, 8) if qb == 0 else slice(0, 4)
                    for a in range(G):
                        sa = (a + qa) % G
                        if qb == 0:
                            # sb == b: full (bh, bl) block copy
                            moves = [((FULL, FULL), (FULL, FULL))]
                        else:
                            # sb = b + 1 mod 4 decomposed on (bh, bl)
                            moves = [((FULL, slice(0, 1)), (FULL, slice(1, 2))),
                                     ((slice(0, 1), slice(1, 2)), (slice(1, 2), slice(0, 1))),
                                     ((slice(1, 2), slice(1, 2)), (slice(0, 1), slice(0, 1)))]
                        for (dbh, dbl), (sbh, sbl) in moves:
                            for ct in range(KC):
                                eng = (nc.gpsimd, nc.vector)[(a + ct) % 2]
                                if fwd:
                                    eng.tensor_copy(dv[:, a, dbh, ct, dbl, di, dj],
                                                    sv[:, sa, sbh, ct, sbl, si, sj])
                                else:
                                    eng.tensor_copy(dv[:, sa, sbh, ct, sbl, si, sj],
                                                    sv[:, a, dbh, ct, dbl, di, dj])

        # ---------- layers ----------
        for i in range(D):
            if _STAGE < 1:
                break
            shift = (i % 2) == 1
            # LN1 -> h (bf16)
            h = hp.tile([P, NT, C], BF16, name="h")
            layernorm(h, x)
            hT_A = hTp.tile([P, NT, KC, P], BF16, name="hT")
            transpose_to(hT_A, h)
            if shift:
                hT = hTp.tile([P, NT, KC, P], BF16, name="hT")
                permute(hT, hT_A, True)
            else:
                hT = hT_A

            # qk + v
            if _STAGE < 2:
                continue
            wqk_sb = wqkp.tile([P, KC, 2 * C], BF16, name="wqk")
            nc.sync.dma_start(wqk_sb[:], dr['wqk'][i].rearrange("(k p) m -> p k m", p=P))
            bqk_sb = bp.tile([P, 8], F32, name="bqk", tag="bqk")
            nc.sync.dma_start(bqk_sb[:], dr['bqk'][i])
            qkT = qkp.tile([P, 8, L], BF16, name="qkT")
            wvp_sb = wvpp.tile([P, KC, VW], BF16, name="wvp")
            nc.sync.dma_start(wvp_sb[:], dr['wvp'][i].rearrange("(k p) m -> p k m", p=P))
            vb_sb = bp.tile([P, VW], F32, name="vb", tag="vb")
            nc.sync.dma_start(vb_sb[:], dr['vb'][i])
            v_aug = vp.tile([P, NT, VW], BF16, name="vaug")
            with tc.tile_pool(name="mmps1", bufs=2, space="PSUM") as mmps, \
                 tc.tile_pool(name="vps", bufs=2, space="PSUM") as vps:
                for mo in range(8):
                    ps = mmps.tile([P, L], F32, name="mm")
                    for kk in range(KC):
                        for tc2 in range(2):
                            nc.tensor.matmul(ps[:, tc2 * 512:(tc2 + 1) * 512],
                                             wqk_sb[:, kk, mo * P:(mo + 1) * P],
                                             hT[:, tc2 * 4:(tc2 + 1) * 4, kk, :],
                                             start=(kk == 0), stop=(kk == KC - 1),
                                             skip_group_check=True)
                    nc.scalar.activation(qkT[:, mo], ps[:], AF.Identity,
                                         bias=bqk_sb[:, mo:mo + 1])
                for tt in range(NT if _STAGE >= 3 else 0):
                    psv = vps.tile([P, VW], F32, name="vps")
                    for kk in range(KC):
                        nc.tensor.matmul(psv[:, 0:512], hT[:, tt, kk, :],
                                         wvp_sb[:, kk, 0:512], start=(kk == 0),
                                         stop=(kk == KC - 1), skip_group_check=True)
                        nc.tensor.matmul(psv[:, 512:VW], hT[:, tt, kk, :],
                                         wvp_sb[:, kk, 512:VW], start=(kk == 0),
                                         stop=(kk == KC - 1), skip_group_check=True)
                    nc.vector.tensor_add(v_aug[:, tt], psv[:], vb_sb[:])

            if _STAGE < 4:
                continue
            # attention: S + softmax + AV
            att = attp.tile([P, NT, C], BF16, name="att")
            with tc.tile_pool(name="sps", bufs=1, space="PSUM") as sps, \
                 tc.tile_pool(name="avps", bufs=1, space="PSUM") as avps:
                for wp2 in range(NT):
                    abt = abp.tile([P, 16 * NW], BF16, name=f"ab{wp2 % 3}")
                    nc.sync.dma_start(abt[:], dr['ab'][i, wp2])
                    pts = []
                    for rg in range(4):
                        sp = sps.tile([P, 4, NW], F32, name=f"s{rg}", tag=f"s{rg}")
                        spf = sp[:].rearrange("p a b -> p (a b)")
                        nc.tensor.matmul(spf, ident_b[:],
                                         abt[:, rg * 256:(rg + 1) * 256],
                                         start=True, stop=False, skip_group_check=True)
                        for hi in range(4):
                            for w01 in range(2):
                                qs = qkT[rg * HD:(rg + 1) * HD, hi,
                                         (wp2 * 2 + w01) * NW:(wp2 * 2 + w01 + 1) * NW]
                                ks = qkT[rg * HD:(rg + 1) * HD, 4 + hi,
                                         (wp2 * 2 + w01) * NW:(wp2 * 2 + w01 + 1) * NW]
                                nc.tensor.matmul(sp[w01 * NW:(w01 + 1) * NW, hi, :],
                                                 ks, qs, start=False,
                                                 stop=(hi == 3 and w01 == 1),
                                                 tile_position=(rg * HD, w01 * NW),
                                                 skip_group_check=True)
                        pt = ppool.tile([P, 4, NW], BF16, name=f"p{rg}", tag=f"p{rg}")
                        nc.scalar.activation(pt[:].rearrange("p a b -> p (a b)"),
                                             spf, AF.Exp)
                        pts.append(pt)
                    for w01 in range(2):
                        rows = slice(w01 * NW, (w01 + 1) * NW)
                        rec = recp.tile([P, HEADS], F32, name=f"rec{w01}", tag=f"rec{w01}")
                        for half in range(2):
                            av = avps.tile([P, 8, VBLK], F32, name=f"av{w01}{half}",
                                           tag=f"av{w01}{half}")
                            for hh in range(8):
                                hglob = half * 8 + hh
                                hi, rg = hglob // 4, hglob % 4
                                nc.tensor.matmul(
                                    av[rows, hh, :], pts[rg][rows, hi, :],
                                    v_aug[rows, wp2, hglob * VBLK:(hglob + 1) * VBLK],
                                    start=True, stop=True,
                                    tile_position=(w01 * NW, w01 * NW))
                            nc.vector.reciprocal(rec[rows, half * 8:(half + 1) * 8],
                                                 av[rows, :, HD])
                            rb = rec[rows, half * 8:(half + 1) * 8] \
                                .rearrange("p (a b) -> p a b", b=1).to_broadcast((NW, 8, HD))
                            dst = att[rows, wp2, half * 256:(half + 1) * 256] \
                                .rearrange("p (a b) -> p a b", b=HD)
                            nc.vector.tensor_mul(dst, av[rows, :, 0:HD], rb)

            # attn transpose back (+ inverse shift permute), proj + residual
            if _STAGE < 5:
                continue
            aT_B = hTp.tile([P, NT, KC, P], BF16, name="hT")
            transpose_to(aT_B, att)
            if shift:
                aT = hTp.tile([P, NT, KC, P], BF16, name="hT")
                permute(aT, aT_B, False)
            else:
                aT = aT_B
            wp_sb = wpp.tile([P, KC, C], BF16, name="wp")
            nc.sync.dma_start(wp_sb[:], dr['wp'][i].rearrange("(k p) m -> p k m", p=P))
            if has_bias:
                pbr_sb = bp.tile([1, C], BF16, name="pbr", tag="pbr")
                nc.sync.dma_start(pbr_sb[:], dr['pbr'][i])
            with tc.tile_pool(name="mmps2", bufs=2, space="PSUM") as mmps:
                for tt in range(NT):
                    ps = mmps.tile([P, C], F32, name="mm")
                    if has_bias:
                        nc.tensor.matmul(ps[:], ones_col[:], pbr_sb[:],
                                         start=True, stop=False, skip_group_check=True)
                    for kk in range(KC):
                        nc.tensor.matmul(ps[:], aT[:, tt, kk, :],
                                         wp_sb[:, kk, :],
                                         start=(kk == 0 and not has_bias),
                                         stop=(kk == KC - 1), skip_group_check=True)
                    nc.vector.tensor_add(x[:, tt], ps[:], x[:, tt])

            # LN2 + h2T
            if _STAGE < 6:
                continue
            h2 = hp.tile([P, NT, C], BF16, name="h")
            layernorm(h2, x)
            h2T = hTp.tile([P, NT, KC, P], BF16, name="hT")
            transpose_to(h2T, h2)

            # MLP
            f1b_sb = bp.tile([P, FH // P], F32, name="f1b", tag="f1b")
            nc.sync.dma_start(f1b_sb[:], dr['f1b'][i])
            if has_bias:
                f2br_sb = bp.tile([1, C], BF16, name="f2br", tag="f2br")
                nc.sync.dma_start(f2br_sb[:], dr['f2br'][i])
            with tc.tile_pool(name="mmps3", bufs=2, space="PSUM") as mmps, \
                 tc.tile_pool(name="fc2ps", bufs=1, space="PSUM") as fc2ps:
                for tc2 in range(2):
                    pso = [fc2ps.tile([P, C], F32, name=f"fc2_{j}", tag=f"fc2_{j}") for j in range(4)]
                    if has_bias:
                        for j in range(4):
                            nc.tensor.matmul(pso[j][:], ones_col[:], f2br_sb[:],
                                             start=True, stop=False, skip_group_check=True)
                    for ho4 in range(4):
                        f1c = f1p.tile([P, KC, 4, P], BF16, name="f1c")
                        nc.sync.dma_start(f1c[:], dr['f1'][i][:, ho4 * 512:(ho4 + 1) * 512]
                                            .rearrange("(k p) (h m) -> p k h m", p=P, m=P))
                        f2c = f2p.tile([P, 4, C], BF16, name="f2c")
                        nc.sync.dma_start(f2c[:], dr['f2'][i][ho4 * 512:(ho4 + 1) * 512, :]
                                            .rearrange("(h p) m -> p h m", p=P))
                        for hs in range(4):
                            ho = ho4 * 4 + hs
                            ps1 = mmps.tile([P, C], F32, name="mm")
                            for kk in range(KC):
                                nc.tensor.matmul(ps1[:], f1c[:, kk, hs, :],
                                                 h2T[:, tc2 * 4:(tc2 + 1) * 4, kk, :],
                                                 start=(kk == 0), stop=(kk == KC - 1))
                            g = gp.tile([P, C], BF16, name="g")
                            nc.scalar.activation(g[:], ps1[:], AF.Gelu, bias=f1b_sb[:, ho:ho + 1])
                            for j in range(4):
                                nc.tensor.matmul(pso[j][:], g[:, j * P:(j + 1) * P], f2c[:, hs, :],
                                                 start=(ho == 0 and not has_bias),
                                                 stop=(ho == FH // P - 1),
                                                 skip_group_check=True)
                    for j in range(4):
                        tt = tc2 * 4 + j
                        nc.vector.tensor_add(x[:, tt], pso[j][:], x[:, tt])

        # ---------- final LN + gelu + pred ----------
        hf = hp.tile([P, NT, C], BF16, name="h")
        layernorm(hf, x)
        nfw_sb = bp.tile([P, KC], F32, name="nfw", tag="nfw")
        nc.sync.dma_start(nfw_sb[:], dr['nfw'])
        nfb_sb = bp.tile([P, KC], F32, name="nfb", tag="nfb")
        nc.sync.dma_start(nfb_sb[:], dr['nfb'])
        pwb_sb = bp.tile([P, N_E // P], F32, name="pwb", tag="pwb", bufs=1)
        nc.sync.dma_start(pwb_sb[:], dr['pwb'])
        gT_pre = hTp.tile([P, NT, KC, P], BF16, name="hT")
        transpose_to(gT_pre, hf)
        gT = hTp.tile([P, NT, KC, P], BF16, name="hT")
        for ct in range(KC):
            nc.scalar.activation(gT[:, :, ct, :], gT_pre[:, :, ct, :], AF.Gelu,
                                 bias=nfb_sb[:, ct:ct + 1], scale=nfw_sb[:, ct:ct + 1])
        with tc.tile_pool(name="mmpsf", bufs=2, space="PSUM") as mmps:
            for no in range(N_E // P):
                pwc = pwp.tile([P, KC, P], BF16, name="pwc")
                nc.sync.dma_start(pwc[:], dr['pw'][:, no * P:(no + 1) * P]
                                  .rearrange("(k p) m -> p k m", p=P))
                ps = mmps.tile([P, L], F32, name="mm")
                for kk in range(KC):
                    for tc2 in range(2):
                        nc.tensor.matmul(ps[:, tc2 * 512:(tc2 + 1) * 512], pwc[:, kk, :],
                                         gT[:, tc2 * 4:(tc2 + 1) * 4, kk, :],
                                         start=(kk == 0), stop=(kk == KC - 1),
                                         skip_group_check=True)
                osb = outp.tile([P, L], BF16, name="osb")
                if no % 2 == 0:
                    nc.scalar.activation(osb[:], ps[:], AF.Identity,
                                         bias=pwb_sb[:, no:no + 1])
                else:
                    nc.vector.tensor_scalar_add(osb[:], ps[:], pwb_sb[:, no:no + 1])
                nc.sync.dma_start(outT[no * P:(no + 1) * P, :], osb[:])

    nc.compile()
    _BUILD_CACHE[key] = nc
    return nc


LAST_RESULTS = None


def kernel(**inputs):
    global LAST_RESULTS
    from concourse import bass_utils
    sh, xts = _prepare(inputs)
    nc = _build(_HAS_BIAS)
    in_maps = []
    for c in range(_NCORES):
        m = dict(sh)
        m['xT'] = xts[c % B]
        in_maps.append(m)
    trace = os.environ.get("BT_TRACE", "0") == "1"
    if trace:
        try:
            import antenv.axon_hooks  # noqa: F401
        except ImportError:
            trace = False
    res = bass_utils.run_bass_kernel_spmd(nc, in_maps, core_ids=list(range(_NCORES)),
                                          trace=trace)
    LAST_RESULTS = res
    outs = []
    for c in range(B):
        oT = np.asarray(res.results[c % _NCORES]['outT'], dtype=np.float32)  # [N_E, L]
        o = oT.T[_WM_INV]                      # [L, N_E] raster order
        outs.append(o)
    return np.stack(outs).astype(np.float32)
